# revision 9
# baseline (speedup 1.0000x reference)
"""CTC loss (T=512, B=32, C=8000, L=2, blank=0) on 8 Trainium2 NeuronCores.

Data-parallel over batch: each core takes 4 sequences. v3 restructure:
  - host packs the class window logit[:, b, 0:20] as one contiguous
    [80, 512] tile X (row = b*20+cls, col = t) -> single fast DMA,
  - host builds a +-1 extraction matrix Wm [80, 100] from targets; ONE
    f32r PE matmul Wm^T @ X yields all 16 streams [slot, t] directly
    (slots at 32-aligned partition groups: d1 rows 0-3, d2 rows 32-35,
    y1 rows 64-67, a rows 96-99),
  - ONE fused inclusive cumsum over the streams (PBX [100, 513]),
  - s1+s3 batched [36,512] (rev rows via tensor_scalar suffix view),
  - s2 stage [4,512], combine, EZ exp with accum row-sums,
  - device ships OUT [36,4] = (SZ, negMz, nm1, nm2); host does the
    final ln, max-unwinding, logaddexp, /L and batch mean in float64.

Notation (per sequence b, t = 0..511):
  a_t = logit[t,b,0], y1_t = logit[t,b,t1], y2_t = logit[t,b,t2]
  P1a_t = sum_{tau<t}(a-y1);  P1brev_c = sum_{t>511-c}(a-y2)
  W1 = ln cumsum exp(P1 - m1) + K;  P2rev_c = W1brev_{c-1} - P1brev_c
  W2 = ln cumsum exp(P2rev - m2) + K
  thr:  Zt_t = ZC_t + W2true_t       skip: Zs_t = ZC_{t+1} + P2true_t
  pcat0 = lnSZt + Mzt + m1a+m1b+m2 - 3K
  pcat1 = lnSZs + Mzs + m1a+m1b - 2K + skip
  loss_b = -logaddexp(pcat0, pcat1)/L
"""
import numpy as np

T = 512
B = 32
C = 8000
L = 2
NCORES = 8
BS = B // NCORES          # 4 sequences per core
CW = 20                   # class window: targets in [1,20), blank=0
NP = 100                  # stream partition span (groups at 0/32/64/96)
NZ = 36                   # two-group span (rows 0-3 and 32-35)
NEG = -1e30
EPS = 4.4e-20   # bottom edge of the HW Ln table's accurate range
KLN = 3e16      # scale so S*KLN spans the Ln-accurate domain
KAPPA = float(np.log(3e16))


def build_bass(dbg=False):
    import concourse.bass as bass
    import concourse.bacc as bacc
    import concourse.mybir as mybir
    import concourse.tile as tile
    from contextlib import ExitStack

    f32 = mybir.dt.float32
    f32r = mybir.dt.float32r
    AT = mybir.ActivationFunctionType
    OP = mybir.AluOpType
    AX = mybir.AxisListType

    nc = bacc.Bacc("TRN2", target_bir_lowering=False, debug=False,
                   num_devices=NCORES)

    # Exp and Ln share the natural_log_exp_and_others ACT table set; pin
    # the chooser there so the table loads once (no Exp<->Ln reloads).
    import types
    from concourse.hw_specs import get_activation_tables

    def _act_loads_one_set(self):
        has_activation = any(isinstance(i, mybir.InstActivation)
                             for b in self.main_func.blocks
                             for i in b.instructions)
        if not has_activation:
            return
        tables = [(n, (fns if n == "natural_log_exp_and_others" else set()))
                  for n, fns in get_activation_tables(self.m.arch).items()]
        bacc._bass_rust.insert_act_table_loads(self, tables)

    nc.insert_act_table_loads = types.MethodType(_act_loads_one_set, nc)

    x_ext = nc.dram_tensor("x", [BS * CW, T], f32r, kind="ExternalInput")
    w_ext = nc.dram_tensor("w", [BS * CW, NP], f32r, kind="ExternalInput")
    out_ext = nc.dram_tensor("out", [NZ, 4], f32, kind="ExternalOutput")

    def dbg_dump(name, ap_):
        if dbg:
            dt = nc.dram_tensor("dbg_" + name, list(ap_.shape), f32,
                                kind="ExternalOutput")
            nc.sync.dma_start(out=dt[:], in_=ap_)

    with tile.TileContext(nc) as tc, ExitStack() as ctx:
        pool = ctx.enter_context(tc.tile_pool(name="p", bufs=1))
        ppool = ctx.enter_context(tc.tile_pool(name="ps", bufs=1, space="PSUM"))

        # ---------- DMAs first ----------
        Xs = pool.tile([BS * CW, T], f32r)
        nc.sync.dma_start(out=Xs[:, 0:256], in_=x_ext[:, 0:256])
        nc.scalar.dma_start(out=Xs[:, 256:512], in_=x_ext[:, 256:512])
        Ws = pool.tile([BS * CW, NP], f32r)
        nc.sync.dma_start(out=Ws[:], in_=w_ext[:])

        # ---------- constants + memsets (off critical path) ----------
        zeros = pool.tile([NP, 1], f32)
        nc.gpsimd.memset(zeros[:], 0.0)
        eps36 = pool.tile([NZ, 1], f32)
        nc.gpsimd.memset(eps36[:], EPS)

        PBX = pool.tile([NP, T + 1], f32)
        nc.gpsimd.memset(PBX[:, 0:1], 0.0)
        P1 = pool.tile([NZ, T], f32)
        nc.gpsimd.memset(P1[32:36, 0:1], 0.0)
        P2 = pool.tile([BS, T], f32)
        nc.gpsimd.memset(P2[:, 0:1], NEG)
        Z = pool.tile([NZ, T], f32)
        nc.gpsimd.memset(Z[0:BS, 0:1], NEG)
        nc.gpsimd.memset(Z[32:36, T - 1:T], NEG)

        # preload the Exp/Ln ACT table during the DMA window
        warm = pool.tile([1, 1], f32)
        nc.scalar.activation(warm[:], zeros[0:1, :], AT.Exp,
                             bias=eps36[0:1, :], scale=1.0)

        # ---------- phase A: one matmul -> 16 streams [slot, t] ----------
        STR = ppool.tile([NP, T], f32, tag="STR")
        nc.tensor.matmul(STR[:], Ws[:], Xs[:], start=True, stop=True)

        # ---------- phase B: ONE fused inclusive cumsum ----------
        # PBX[r, t+1] = sum_{tau<=t} STR[r, tau]; col 0 = 0; col 512 = total.
        nc.vector.tensor_tensor_scan(
            PBX[:, 1:T + 1], STR[:, 0:T],
            zeros[:].broadcast_to((NP, T)), 0.0,
            op0=OP.add, op1=OP.bypass)

        # P1 rows 0-3 = P1a (excl cumsum d1); rows 32-35 = P1brev (suffix d2)
        nc.vector.tensor_copy(P1[0:BS, :], PBX[0:BS, 0:T])
        nc.vector.tensor_scalar(P1[32:36, 1:T],
                                PBX[32:36, 1:T][:, ::-1],
                                PBX[32:36, T:T + 1], -1.0,
                                op0=OP.subtract, op1=OP.mult)

        # ---------- stage s1 (rows 0-3) + s3 (rows 32-35, rev) ----------
        nm1 = pool.tile([NZ, 1], f32)
        nc.vector.tensor_reduce(nm1[:], P1[:], axis=AX.X, op=OP.max,
                                negate=True)
        E1 = ppool.tile([NZ, T], f32, tag="E1")
        nc.scalar.activation(E1[:], P1[:], AT.Exp, bias=nm1[:], scale=1.0)
        # off-path fills: demoted so the scheduler slots them into gaps
        TAs = pool.tile([BS, T], f32)     # TAs_t = Atot - Ae_t (cols 1..511)
        TY1z = pool.tile([BS, T], f32)    # TY1e at base partition 0
        with tc.high_priority(offset=-10000):
            nc.gpsimd.tensor_scalar(TAs[:, 1:T], PBX[96:100, 1:T],
                                    PBX[96:100, T:T + 1], -1.0,
                                    op0=OP.subtract, op1=OP.mult)
            nc.gpsimd.tensor_copy(TY1z[:], PBX[64:68, 0:T])
        S1 = pool.tile([NZ, T], f32)
        nc.vector.tensor_tensor_scan(S1[:], E1[:],
                                     zeros[0:NZ, :].broadcast_to((NZ, T)),
                                     0.0, op0=OP.add, op1=OP.bypass)
        W1 = pool.tile([NZ, T], f32)      # W' = true W + KAPPA
        nc.scalar.activation(W1[:], S1[:], AT.Ln, bias=eps36[:], scale=KLN)

        # ---------- stage s2 (rev) ----------
        nc.vector.tensor_tensor(P2[:, 1:T], W1[32:36, 0:T - 1],
                                P1[32:36, 1:T], op=OP.subtract)
        nm2 = pool.tile([BS, 1], f32)
        nc.vector.tensor_reduce(nm2[:], P2[:], axis=AX.X, op=OP.max,
                                negate=True)
        E2 = ppool.tile([BS, T], f32, tag="E2")
        nc.scalar.activation(E2[:], P2[:], AT.Exp, bias=nm2[:], scale=1.0)
        # skip half of Z + ZA/ZC fills hide under the s2 window
        OUT = pool.tile([NZ, 4], f32)
        ZAp = pool.tile([BS, T], f32)     # ZA'_t = TY1_{t-1} + W1a_{t-1}
        ZC = pool.tile([BS, T], f32)      # ZC_t = ZA'_t + TAs_t
        with tc.high_priority(offset=-10000):
            nc.gpsimd.tensor_copy(OUT[:, 2:3], nm1[:])
            nc.gpsimd.tensor_copy(OUT[0:BS, 3:4], nm2[:])
            nc.gpsimd.tensor_tensor(ZAp[:, 1:T], TY1z[:, 1:T],
                                    W1[0:BS, 0:T - 1], op=OP.add)
            nc.gpsimd.tensor_tensor(ZC[:, 1:T], ZAp[:, 1:T],
                                    TAs[:, 1:T], op=OP.add)
            nc.gpsimd.tensor_tensor(Z[32:36, 0:T - 1], ZC[:, 1:T],
                                    P2[:, 1:T][:, ::-1], op=OP.add)
        S2 = pool.tile([BS, T], f32)
        nc.vector.tensor_tensor_scan(S2[:], E2[:],
                                     zeros[0:BS, :].broadcast_to((BS, T)),
                                     0.0, op0=OP.add, op1=OP.bypass)
        W2 = pool.tile([BS, T], f32)      # W' = true W + KAPPA
        nc.scalar.activation(W2[:], S2[:], AT.Ln, bias=eps36[0:BS, :],
                             scale=KLN)

        # ---------- combine: thr half, global max, EZ ----------
        nc.vector.tensor_tensor(Z[0:BS, 1:T], ZC[:, 1:T],
                                W2[:, 0:T - 1][:, ::-1], op=OP.add)
        negMz = pool.tile([NZ, 1], f32)
        nc.vector.tensor_reduce(negMz[:], Z[:], axis=AX.X, op=OP.max,
                                negate=True)
        nc.vector.tensor_copy(OUT[:, 1:2], negMz[:])
        EZ = ppool.tile([NZ, T], f32, tag="EZ")
        nc.scalar.activation(EZ[:], Z[:], AT.Exp, bias=negMz[:], scale=1.0,
                             accum_out=OUT[:, 0:1])
        nc.sync.dma_start(out=out_ext[:], in_=OUT[:])

    nc.compile()
    return nc


def make_in_maps(logit, targets):
    logit = np.asarray(logit, dtype=np.float32)
    targets = np.asarray(targets)
    in_maps = []
    for core in range(NCORES):
        bsl = slice(core * BS, (core + 1) * BS)
        # [T, BS, CW] -> [BS, CW, T] -> [BS*CW, T] contiguous
        x = np.ascontiguousarray(
            logit[:, bsl, 0:CW].transpose(1, 2, 0)).reshape(BS * CW, T)
        tg = targets[bsl]
        w = np.zeros((BS * CW, NP), np.float32)
        for b in range(BS):
            t1, t2 = int(tg[b, 0]), int(tg[b, 1])
            w[b * CW + 0, 0 + b] += 1.0      # d1 = a - y1
            w[b * CW + t1, 0 + b] -= 1.0
            w[b * CW + 0, 32 + b] += 1.0     # d2 = a - y2
            w[b * CW + t2, 32 + b] -= 1.0
            w[b * CW + t1, 64 + b] += 1.0    # y1
            w[b * CW + 0, 96 + b] += 1.0     # a
        in_maps.append({"x": x, "w": w})
    return in_maps


def finish(results, targets):
    """Host gather: per-core OUT [36,4] -> per-seq losses [32] (float64)."""
    targets = np.asarray(targets)
    losses = np.empty(B, np.float64)
    for core, r in enumerate(results):
        o = np.asarray(r["out"], np.float64)     # [NZ, 4]
        sz_t, sz_s = o[0:BS, 0], o[32:36, 0]
        mz_t, mz_s = -o[0:BS, 1], -o[32:36, 1]
        m1a, m1b = -o[0:BS, 2], -o[32:36, 2]
        m2 = -o[0:BS, 3]
        tg = targets[core * BS:(core + 1) * BS]
        skip = np.where(tg[:, 0] != tg[:, 1], 0.0, NEG)
        pcat0 = np.log(sz_t) + mz_t + m1a + m1b + m2 - 3 * KAPPA
        pcat1 = np.log(sz_s) + mz_s + m1a + m1b - 2 * KAPPA + skip
        losses[core * BS:(core + 1) * BS] = \
            -np.logaddexp(pcat0, pcat1) / L
    return losses


_CACHED = {}


def kernel(logit, label, targets):
    from concourse.bass_utils import run_bass_kernel_spmd
    if "nc" not in _CACHED:
        _CACHED["nc"] = build_bass()
    nc = _CACHED["nc"]
    in_maps = make_in_maps(logit, targets)
    res = run_bass_kernel_spmd(nc, in_maps, core_ids=list(range(NCORES)))
    losses = finish(res.results, targets)
    return np.float32(losses.mean())


# revision 14
# speedup vs baseline: 1.0223x; 1.0223x over previous
"""CTC loss (T=512, B=32, C=8000, L=2, blank=0) on 8 Trainium2 NeuronCores.

Data-parallel over batch: each core takes 4 sequences. v4 restructure:
  - host packs the class window logit[:, b, 0:20] plus time-reversed
    a / y2 rows as one contiguous [88, 512] tile X -> 4-queue DMA,
  - host builds a +-1 extraction matrix Wm [88, 100] from targets; two
    f32r PE matmuls (cols 0:256 / 256:512) yield all streams [slot, t]:
    d1 rows 0-3, d2rev rows 32-35, y1 rows 64-67, a rows 96-99,
  - ONE fused inclusive cumsum (PBX [100, 513]); because d2 arrives
    time-reversed, PBX[32:36, 0:512] IS the suffix-sum P1brev and
    PBX[0:4, 0:512] IS P1a - no copies, no reversed tensor_scalar,
  - s1+s3 batched [36,512] as a direct view of PBX,
  - s2 stage [4,512] with tensor_tensor_reduce-fused max,
  - combine with TTR-fused maxes; EZ exp accumulates row sums into OUT,
  - device ships OUT [36,4] = (SZ, negMz, nm1, nm2); host does the
    final ln, max-unwinding, logaddexp, /L and batch mean in float64.

Notation (per sequence b, t = 0..511):
  a_t = logit[t,b,0], y1_t = logit[t,b,t1], y2_t = logit[t,b,t2]
  P1a_t = sum_{tau<t}(a-y1);  P1brev_c = sum_{t>511-c}(a-y2)
  W1 = ln cumsum exp(P1 - m1) + K;  P2rev_c = W1brev_{c-1} - P1brev_c
  W2 = ln cumsum exp(P2rev - m2) + K
  thr:  Zt_t = ZC_t + W2true_t       skip: Zs_t = ZC_{t+1} + P2true_t
  pcat0 = lnSZt + Mzt + m1a+m1b+m2 - 3K
  pcat1 = lnSZs + Mzs + m1a+m1b - 2K + skip
  loss_b = -logaddexp(pcat0, pcat1)/L
"""
import numpy as np

T = 512
B = 32
C = 8000
L = 2
NCORES = 8
BS = B // NCORES          # 4 sequences per core
CW = 20                   # class window: targets in [1,20), blank=0
XR = BS * CW + 2 * BS     # 88 input rows (80 fwd + 4 a_rev + 4 y2_rev)
NP = 100                  # stream partition span (groups at 0/32/64/96)
NZ = 36                   # two-group span (rows 0-3 and 32-35)
NEG = -1e30
EPS = 4.4e-20   # bottom edge of the HW Ln table's accurate range
KLN = 3e16      # scale so S*KLN spans the Ln-accurate domain
KAPPA = float(np.log(3e16))


def build_bass(dbg=False):
    import concourse.bass as bass
    import concourse.bacc as bacc
    import concourse.mybir as mybir
    import concourse.tile as tile
    from contextlib import ExitStack

    f32 = mybir.dt.float32
    f32r = mybir.dt.float32r
    AT = mybir.ActivationFunctionType
    OP = mybir.AluOpType
    AX = mybir.AxisListType

    nc = bacc.Bacc("TRN2", target_bir_lowering=False, debug=False,
                   num_devices=NCORES)

    # Exp and Ln share the natural_log_exp_and_others ACT table set; pin
    # the chooser there so the table loads once (no Exp<->Ln reloads).
    import types
    from concourse.hw_specs import get_activation_tables

    def _act_loads_one_set(self):
        has_activation = any(isinstance(i, mybir.InstActivation)
                             for b in self.main_func.blocks
                             for i in b.instructions)
        if not has_activation:
            return
        tables = [(n, (fns if n == "natural_log_exp_and_others" else set()))
                  for n, fns in get_activation_tables(self.m.arch).items()]
        bacc._bass_rust.insert_act_table_loads(self, tables)

    nc.insert_act_table_loads = types.MethodType(_act_loads_one_set, nc)

    x_ext = nc.dram_tensor("x", [XR, T], f32r, kind="ExternalInput")
    w_ext = nc.dram_tensor("w", [XR, NP], f32r, kind="ExternalInput")
    out_ext = nc.dram_tensor("out", [NZ, 4], f32, kind="ExternalOutput")

    def dbg_dump(name, ap_):
        if dbg:
            dt = nc.dram_tensor("dbg_" + name, list(ap_.shape), f32,
                                kind="ExternalOutput")
            nc.sync.dma_start(out=dt[:], in_=ap_)

    with tile.TileContext(nc) as tc, ExitStack() as ctx:
        pool = ctx.enter_context(tc.tile_pool(name="p", bufs=1))
        ppool = ctx.enter_context(tc.tile_pool(name="ps", bufs=1, space="PSUM"))

        # ---------- DMAs first: W + 4 X quarters on 4 engine queues ----
        Xs = pool.tile([XR, T], f32r)
        Ws = pool.tile([XR, NP], f32r)
        nc.sync.dma_start(out=Ws[:], in_=w_ext[:])
        nc.scalar.dma_start(out=Xs[:, 0:256], in_=x_ext[:, 0:256])
        nc.sync.dma_start(out=Xs[:, 256:512], in_=x_ext[:, 256:512])

        # ---------- constants + memsets (off critical path) ----------
        zeros = pool.tile([NP, 1], f32)
        nc.gpsimd.memset(zeros[:], 0.0)
        eps36 = pool.tile([NZ, 1], f32)
        nc.gpsimd.memset(eps36[:], EPS)

        PBX = pool.tile([NP, T + 1], f32)
        nc.gpsimd.memset(PBX[:, 0:1], 0.0)
        P2 = pool.tile([BS, T], f32)
        nc.gpsimd.memset(P2[:, 0:1], NEG)
        Z = pool.tile([NZ, T], f32)
        nc.gpsimd.memset(Z[0:BS, 0:1], NEG)
        nc.gpsimd.memset(Z[32:36, T - 1:T], NEG)

        # preload the Exp/Ln ACT table during the DMA window
        warm = pool.tile([1, 1], f32)
        nc.scalar.activation(warm[:], zeros[0:1, :], AT.Exp,
                             bias=eps36[0:1, :], scale=1.0)

        # ---------- phase A: two matmuls -> streams [slot, t] ----------
        STR = ppool.tile([NP, T], f32, tag="STR")
        nc.tensor.matmul(STR[:, 0:256], Ws[:], Xs[:, 0:256],
                         start=True, stop=True)
        nc.tensor.matmul(STR[:, 256:512], Ws[:], Xs[:, 256:512],
                         start=True, stop=True)

        # ---------- phase B: ONE fused inclusive cumsum ----------
        # PBX[r, t+1] = sum_{tau<=t} STR[r, tau]; col 0 = 0.
        # Rows 0-3: P1a = PBX[0:4, 0:512] (exclusive-view). Rows 32-35:
        # d2 arrives time-reversed, so P1brev = PBX[32:36, 0:512].
        nc.vector.tensor_tensor_scan(
            PBX[:, 1:T + 1], STR[:, 0:T],
            zeros[:].broadcast_to((NP, T)), 0.0,
            op0=OP.add, op1=OP.bypass)
        P1v = PBX[0:NZ, 0:T]

        # ---------- stage s1 (rows 0-3) + s3 (rows 32-35, rev) ----------
        OUT = pool.tile([NZ, 4], f32)
        nm1 = pool.tile([NZ, 1], f32)
        nc.vector.tensor_reduce(nm1[:], P1v, axis=AX.X, op=OP.max,
                                negate=True)
        E1 = ppool.tile([NZ, T], f32, tag="E1")
        nc.scalar.activation(E1[:], P1v, AT.Exp, bias=nm1[:], scale=1.0)
        # off-path fills: demoted so the scheduler slots them into the
        # ACT windows on the Vector queue
        TAs = pool.tile([BS, T], f32)     # TAs_t = Atot - Ae_t (cols 1..511)
        TY1z = pool.tile([BS, T], f32)    # TY1e at base partition 0
        ZCp = pool.tile([BS, T], f32)     # TY1e_t + TAs_t
        with tc.high_priority(offset=-10000):
            nc.vector.tensor_scalar(TAs[:, 1:T], PBX[96:100, 1:T],
                                    PBX[96:100, T:T + 1], -1.0,
                                    op0=OP.subtract, op1=OP.mult)
            nc.vector.tensor_copy(TY1z[:], PBX[64:68, 0:T])
            nc.vector.tensor_tensor(ZCp[:, 1:T], TY1z[:, 1:T],
                                    TAs[:, 1:T], op=OP.add)
        S1 = pool.tile([NZ, T], f32)
        nc.vector.tensor_tensor_scan(S1[:], E1[:],
                                     zeros[0:NZ, :].broadcast_to((NZ, T)),
                                     0.0, op0=OP.add, op1=OP.bypass)
        W1 = pool.tile([NZ, T], f32)      # W' = true W + KAPPA
        nc.scalar.activation(W1[:], S1[:], AT.Ln, bias=eps36[:], scale=KLN)

        # ---------- stage s2 (rev), build fused with its max ----------
        m2pos = pool.tile([BS, 1], f32)
        nc.vector.tensor_tensor(P2[:, 1:T], W1[32:36, 0:T - 1],
                                PBX[32:36, 1:T], op=OP.subtract)
        nc.vector.tensor_reduce(m2pos[:], P2[:], axis=AX.X, op=OP.max)
        nm2 = pool.tile([BS, 1], f32)
        nc.vector.tensor_scalar_mul(nm2[:], m2pos[:], -1.0)
        E2 = ppool.tile([BS, T], f32, tag="E2")
        nc.scalar.activation(E2[:], P2[:], AT.Exp, bias=nm2[:], scale=1.0)
        # ZC + skip half of Z hide under the s2 ACT windows
        negMz = pool.tile([NZ, 1], f32)
        ZC = pool.tile([BS, T], f32)      # ZC_t = TY1e_t + TAs_t + W1a_{t-1}
        with tc.high_priority(offset=-10000):
            nc.vector.tensor_tensor(ZC[:, 1:T], ZCp[:, 1:T],
                                    W1[0:BS, 0:T - 1], op=OP.add)
            nc.vector.tensor_tensor(Z[32:36, 0:T - 1], ZC[:, 1:T],
                                    P2[:, 1:T][:, ::-1], op=OP.add)
        S2 = pool.tile([BS, T], f32)
        nc.vector.tensor_tensor_scan(S2[:], E2[:],
                                     zeros[0:BS, :].broadcast_to((BS, T)),
                                     0.0, op0=OP.add, op1=OP.bypass)
        W2 = pool.tile([BS, T], f32)      # W' = true W + KAPPA
        nc.scalar.activation(W2[:], S2[:], AT.Ln, bias=eps36[0:BS, :],
                             scale=KLN)

        # ---------- combine: thr half fused with max, EZ ----------
        nc.vector.tensor_tensor(Z[0:BS, 1:T], ZC[:, 1:T],
                                W2[:, 0:T - 1][:, ::-1], op=OP.add)
        nc.vector.tensor_reduce(negMz[:], Z[:], axis=AX.X, op=OP.max,
                                negate=True)
        with tc.high_priority(offset=-10000):
            nc.vector.tensor_copy(OUT[:, 2:3], nm1[:])
            nc.vector.tensor_copy(OUT[0:BS, 3:4], nm2[:])
        nc.vector.tensor_copy(OUT[:, 1:2], negMz[:])
        EZ = ppool.tile([NZ, T], f32, tag="EZ")
        SZ = pool.tile([NZ, 1], f32)
        nc.scalar.activation(EZ[:], Z[:], AT.Exp, bias=negMz[:], scale=1.0,
                             accum_out=SZ[:])
        nc.vector.tensor_copy(OUT[:, 0:1], SZ[:])
        nc.sync.dma_start(out=out_ext[:], in_=OUT[:])

    nc.compile()
    return nc


def make_in_maps(logit, targets):
    logit = np.asarray(logit, dtype=np.float32)
    targets = np.asarray(targets)
    in_maps = []
    for core in range(NCORES):
        bsl = slice(core * BS, (core + 1) * BS)
        tg = targets[bsl]
        x = np.empty((XR, T), np.float32)
        # rows 0-79: class window, row = b*20 + cls, col = t
        x[0:BS * CW] = np.ascontiguousarray(
            logit[:, bsl, 0:CW].transpose(1, 2, 0)).reshape(BS * CW, T)
        # rows 80-83: a time-reversed; rows 84-87: y2 time-reversed
        for b in range(BS):
            x[BS * CW + b] = logit[::-1, core * BS + b, 0]
            x[BS * CW + BS + b] = logit[::-1, core * BS + b, int(tg[b, 1])]
        w = np.zeros((XR, NP), np.float32)
        for b in range(BS):
            t1 = int(tg[b, 0])
            w[b * CW + 0, 0 + b] += 1.0        # d1 = a - y1
            w[b * CW + t1, 0 + b] -= 1.0
            w[BS * CW + b, 32 + b] += 1.0      # d2rev = a_rev - y2_rev
            w[BS * CW + BS + b, 32 + b] -= 1.0
            w[b * CW + t1, 64 + b] += 1.0      # y1
            w[b * CW + 0, 96 + b] += 1.0       # a
        in_maps.append({"x": x, "w": w})
    return in_maps


def finish(results, targets):
    """Host gather: per-core OUT [36,4] -> per-seq losses [32] (float64)."""
    targets = np.asarray(targets)
    losses = np.empty(B, np.float64)
    for core, r in enumerate(results):
        o = np.asarray(r["out"], np.float64)     # [NZ, 4]
        sz_t, sz_s = o[0:BS, 0], o[32:36, 0]
        mz_t, mz_s = -o[0:BS, 1], -o[32:36, 1]
        m1a, m1b = -o[0:BS, 2], -o[32:36, 2]
        m2 = -o[0:BS, 3]
        tg = targets[core * BS:(core + 1) * BS]
        skip = np.where(tg[:, 0] != tg[:, 1], 0.0, NEG)
        pcat0 = np.log(sz_t) + mz_t + m1a + m1b + m2 - 3 * KAPPA
        pcat1 = np.log(sz_s) + mz_s + m1a + m1b - 2 * KAPPA + skip
        losses[core * BS:(core + 1) * BS] = \
            -np.logaddexp(pcat0, pcat1) / L
    return losses


_CACHED = {}


def kernel(logit, label, targets):
    from concourse.bass_utils import run_bass_kernel_spmd
    if "nc" not in _CACHED:
        _CACHED["nc"] = build_bass()
    nc = _CACHED["nc"]
    in_maps = make_in_maps(logit, targets)
    res = run_bass_kernel_spmd(nc, in_maps, core_ids=list(range(NCORES)))
    losses = finish(res.results, targets)
    return np.float32(losses.mean())


# revision 15
# speedup vs baseline: 1.2765x; 1.2486x over previous
"""CTC loss (T=512, B=32, C=8000, L=2, blank=0) on 8 Trainium2 NeuronCores.

Data-parallel over batch: each core takes 4 sequences. v5 structure:
  - host gathers the 16 needed logit streams per core as one contiguous
    [16, 512] tile X (a, y1, a_rev, y2_rev per sequence) -> tiny DMA,
  - ONE f32r PE matmul with a +-1 matrix Wm [16, 100] forms the DP
    streams at 32-aligned partition groups: d1 rows 0-3, d2rev rows
    32-35, y1 rows 64-67, a rows 96-99,
  - ONE fused inclusive cumsum (PBX [100, 513]); because d2 arrives
    time-reversed, PBX[32:36, 0:512] IS the suffix-sum P1brev and
    PBX[0:4, 0:512] IS P1a - no copies, no reversed tensor_scalar,
  - s1+s3 batched [36,512] as a direct view of PBX,
  - Vector carries only the critical chain (scans / maxes / P2 / Zthr);
    GpSimd carries the ZC/Zskip side chain and OUT staging copies,
  - device ships OUT [36,4] = (SZ, negMz, nm1, nm2); host does the
    final ln, max-unwinding, logaddexp, /L and batch mean in float64.

Notation (per sequence b, t = 0..511):
  a_t = logit[t,b,0], y1_t = logit[t,b,t1], y2_t = logit[t,b,t2]
  P1a_t = sum_{tau<t}(a-y1);  P1brev_c = sum_{t>511-c}(a-y2)
  W1 = ln cumsum exp(P1 - m1) + K;  P2rev_c = W1brev_{c-1} - P1brev_c
  W2 = ln cumsum exp(P2rev - m2) + K
  thr:  Zt_t = ZC_t + W2true_t       skip: Zs_t = ZC_{t+1} + P2true_t
  pcat0 = lnSZt + Mzt + m1a+m1b+m2 - 3K
  pcat1 = lnSZs + Mzs + m1a+m1b - 2K + skip
  loss_b = -logaddexp(pcat0, pcat1)/L
"""
import numpy as np

T = 512
B = 32
C = 8000
L = 2
NCORES = 8
BS = B // NCORES          # 4 sequences per core
XR = 4 * BS               # 16 input stream rows
NP = 100                  # stream partition span (groups at 0/32/64/96)
NZ = 36                   # two-group span (rows 0-3 and 32-35)
NEG = -1e30
EPS = 4.4e-20   # bottom edge of the HW Ln table's accurate range
KLN = 3e16      # scale so S*KLN spans the Ln-accurate domain
KAPPA = float(np.log(3e16))


def build_bass(dbg=False):
    import concourse.bass as bass
    import concourse.bacc as bacc
    import concourse.mybir as mybir
    import concourse.tile as tile
    from contextlib import ExitStack

    f32 = mybir.dt.float32
    f32r = mybir.dt.float32r
    AT = mybir.ActivationFunctionType
    OP = mybir.AluOpType
    AX = mybir.AxisListType

    nc = bacc.Bacc("TRN2", target_bir_lowering=False, debug=False,
                   num_devices=NCORES)

    # Exp and Ln share the natural_log_exp_and_others ACT table set; pin
    # the chooser there so the table loads once (no Exp<->Ln reloads).
    import types
    from concourse.hw_specs import get_activation_tables

    def _act_loads_one_set(self):
        has_activation = any(isinstance(i, mybir.InstActivation)
                             for b in self.main_func.blocks
                             for i in b.instructions)
        if not has_activation:
            return
        tables = [(n, (fns if n == "natural_log_exp_and_others" else set()))
                  for n, fns in get_activation_tables(self.m.arch).items()]
        bacc._bass_rust.insert_act_table_loads(self, tables)

    nc.insert_act_table_loads = types.MethodType(_act_loads_one_set, nc)

    x_ext = nc.dram_tensor("x", [XR, T], f32r, kind="ExternalInput")
    w_ext = nc.dram_tensor("w", [XR, NP], f32r, kind="ExternalInput")
    out_ext = nc.dram_tensor("out", [NZ, 4], f32, kind="ExternalOutput")

    with tile.TileContext(nc) as tc, ExitStack() as ctx:
        pool = ctx.enter_context(tc.tile_pool(name="p", bufs=1))
        ppool = ctx.enter_context(tc.tile_pool(name="ps", bufs=1, space="PSUM"))

        # ---------- DMAs first (both tiny and contiguous) ----------
        Xs = pool.tile([XR, T], f32r)
        Ws = pool.tile([XR, NP], f32r)
        nc.sync.dma_start(out=Xs[:], in_=x_ext[:])
        nc.scalar.dma_start(out=Ws[:], in_=w_ext[:])

        # ---------- constants + memsets (off critical path) ----------
        zeros = pool.tile([NP, 1], f32)
        nc.gpsimd.memset(zeros[:], 0.0)
        eps36 = pool.tile([NZ, 1], f32)
        nc.gpsimd.memset(eps36[:], EPS)

        PBX = pool.tile([NP, T + 1], f32)
        nc.gpsimd.memset(PBX[:, 0:1], 0.0)
        P2 = pool.tile([BS, T], f32)
        nc.gpsimd.memset(P2[:, 0:1], NEG)
        Z = pool.tile([NZ, T], f32)
        nc.gpsimd.memset(Z[0:BS, 0:1], NEG)
        nc.gpsimd.memset(Z[32:36, T - 1:T], NEG)

        # preload the Exp/Ln ACT table during the DMA window
        warm = pool.tile([1, 1], f32)
        nc.scalar.activation(warm[:], zeros[0:1, :], AT.Exp,
                             bias=eps36[0:1, :], scale=1.0)

        # ---------- phase A: one matmul -> streams [slot, t] ----------
        STR = ppool.tile([NP, T], f32, tag="STR")
        nc.tensor.matmul(STR[:], Ws[:], Xs[:], start=True, stop=True)

        # ---------- phase B: ONE fused inclusive cumsum ----------
        # PBX[r, t+1] = sum_{tau<=t} STR[r, tau]; col 0 = 0.
        # Rows 0-3: P1a = PBX[0:4, 0:512] (exclusive-view). Rows 32-35:
        # d2 arrives time-reversed, so P1brev = PBX[32:36, 0:512].
        nc.vector.tensor_tensor_scan(
            PBX[:, 1:T + 1], STR[:, 0:T],
            zeros[:].broadcast_to((NP, T)), 0.0,
            op0=OP.add, op1=OP.bypass)
        P1v = PBX[0:NZ, 0:T]

        # ---------- stage s1 (rows 0-3) + s3 (rows 32-35, rev) ----------
        OUT = pool.tile([NZ, 4], f32)
        nm1 = pool.tile([NZ, 1], f32)
        nc.vector.tensor_reduce(nm1[:], P1v, axis=AX.X, op=OP.max,
                                negate=True)
        E1 = ppool.tile([NZ, T], f32, tag="E1")
        nc.scalar.activation(E1[:], P1v, AT.Exp, bias=nm1[:], scale=1.0)
        # TAs fits in the E1-exp window on Vector; the rest of the side
        # chain lives on GpSimd.
        TAs = pool.tile([BS, T], f32)     # TAs_t = Atot - Ae_t (cols 1..511)
        TY1z = pool.tile([BS, T], f32)    # TY1e at base partition 0
        ZCp = pool.tile([BS, T], f32)     # TY1e_t + TAs_t
        nc.vector.tensor_scalar(TAs[:, 1:T], PBX[96:100, 1:T],
                                PBX[96:100, T:T + 1], -1.0,
                                op0=OP.subtract, op1=OP.mult)
        nc.gpsimd.tensor_copy(TY1z[:], PBX[64:68, 0:T])
        nc.gpsimd.tensor_tensor(ZCp[:, 1:T], TY1z[:, 1:T],
                                TAs[:, 1:T], op=OP.add)
        nc.gpsimd.tensor_copy(OUT[:, 2:3], nm1[:])
        S1 = pool.tile([NZ, T], f32)
        nc.vector.tensor_tensor_scan(S1[:], E1[:],
                                     zeros[0:NZ, :].broadcast_to((NZ, T)),
                                     0.0, op0=OP.add, op1=OP.bypass)
        W1 = pool.tile([NZ, T], f32)      # W' = true W + KAPPA
        nc.scalar.activation(W1[:], S1[:], AT.Ln, bias=eps36[:], scale=KLN)

        # ---------- stage s2 (rev) ----------
        m2pos = pool.tile([BS, 1], f32)
        nc.vector.tensor_tensor(P2[:, 1:T], W1[32:36, 0:T - 1],
                                PBX[32:36, 1:T], op=OP.subtract)
        nc.vector.tensor_reduce(m2pos[:], P2[:], axis=AX.X, op=OP.max)
        nm2 = pool.tile([BS, 1], f32)
        nc.vector.tensor_scalar_mul(nm2[:], m2pos[:], -1.0)
        E2 = ppool.tile([BS, T], f32, tag="E2")
        nc.scalar.activation(E2[:], P2[:], AT.Exp, bias=nm2[:], scale=1.0)
        # ZC / Zskip side chain on GpSimd under the s2 windows
        ZC = pool.tile([BS, T], f32)      # ZC_t = TY1e_t + TAs_t + W1a_{t-1}
        nc.gpsimd.tensor_tensor(ZC[:, 1:T], ZCp[:, 1:T],
                                W1[0:BS, 0:T - 1], op=OP.add)
        nc.gpsimd.tensor_tensor(Z[32:36, 0:T - 1], ZC[:, 1:T],
                                P2[:, 1:T][:, ::-1], op=OP.add)
        nc.gpsimd.tensor_copy(OUT[0:BS, 3:4], nm2[:])
        S2 = pool.tile([BS, T], f32)
        nc.vector.tensor_tensor_scan(S2[:], E2[:],
                                     zeros[0:BS, :].broadcast_to((BS, T)),
                                     0.0, op0=OP.add, op1=OP.bypass)
        W2 = pool.tile([BS, T], f32)      # W' = true W + KAPPA
        nc.scalar.activation(W2[:], S2[:], AT.Ln, bias=eps36[0:BS, :],
                             scale=KLN)

        # ---------- combine: thr half, global max, EZ ----------
        nc.vector.tensor_tensor(Z[0:BS, 1:T], ZC[:, 1:T],
                                W2[:, 0:T - 1][:, ::-1], op=OP.add)
        negMz = pool.tile([NZ, 1], f32)
        nc.vector.tensor_reduce(negMz[:], Z[:], axis=AX.X, op=OP.max,
                                negate=True)
        nc.gpsimd.tensor_copy(OUT[:, 1:2], negMz[:])
        EZ = ppool.tile([NZ, T], f32, tag="EZ")
        SZ = pool.tile([NZ, 1], f32)
        nc.scalar.activation(EZ[:], Z[:], AT.Exp, bias=negMz[:], scale=1.0,
                             accum_out=SZ[:])
        nc.vector.tensor_copy(OUT[:, 0:1], SZ[:])
        nc.sync.dma_start(out=out_ext[:], in_=OUT[:])

    nc.compile()
    return nc


def make_in_maps(logit, targets):
    logit = np.asarray(logit, dtype=np.float32)
    targets = np.asarray(targets)
    in_maps = []
    for core in range(NCORES):
        tg = targets[core * BS:(core + 1) * BS]
        x = np.empty((XR, T), np.float32)
        for b in range(BS):
            gb = core * BS + b
            x[0 + b] = logit[:, gb, 0]                      # a
            x[BS + b] = logit[:, gb, int(tg[b, 0])]         # y1
            x[2 * BS + b] = logit[::-1, gb, 0]              # a reversed
            x[3 * BS + b] = logit[::-1, gb, int(tg[b, 1])]  # y2 reversed
        w = np.zeros((XR, NP), np.float32)
        for b in range(BS):
            w[0 + b, 0 + b] = 1.0        # d1 = a - y1
            w[BS + b, 0 + b] = -1.0
            w[2 * BS + b, 32 + b] = 1.0  # d2rev = a_rev - y2_rev
            w[3 * BS + b, 32 + b] = -1.0
            w[BS + b, 64 + b] = 1.0      # y1
            w[0 + b, 96 + b] = 1.0       # a
        in_maps.append({"x": x, "w": w})
    return in_maps


def finish(results, targets):
    """Host gather: per-core OUT [36,4] -> per-seq losses [32] (float64)."""
    targets = np.asarray(targets)
    losses = np.empty(B, np.float64)
    for core, r in enumerate(results):
        o = np.asarray(r["out"], np.float64)     # [NZ, 4]
        sz_t, sz_s = o[0:BS, 0], o[32:36, 0]
        mz_t, mz_s = -o[0:BS, 1], -o[32:36, 1]
        m1a, m1b = -o[0:BS, 2], -o[32:36, 2]
        m2 = -o[0:BS, 3]
        tg = targets[core * BS:(core + 1) * BS]
        skip = np.where(tg[:, 0] != tg[:, 1], 0.0, NEG)
        pcat0 = np.log(sz_t) + mz_t + m1a + m1b + m2 - 3 * KAPPA
        pcat1 = np.log(sz_s) + mz_s + m1a + m1b - 2 * KAPPA + skip
        losses[core * BS:(core + 1) * BS] = \
            -np.logaddexp(pcat0, pcat1) / L
    return losses


_CACHED = {}


def kernel(logit, label, targets):
    from concourse.bass_utils import run_bass_kernel_spmd
    if "nc" not in _CACHED:
        _CACHED["nc"] = build_bass()
    nc = _CACHED["nc"]
    in_maps = make_in_maps(logit, targets)
    res = run_bass_kernel_spmd(nc, in_maps, core_ids=list(range(NCORES)))
    losses = finish(res.results, targets)
    return np.float32(losses.mean())


# revision 17
# speedup vs baseline: 1.2869x; 1.0082x over previous
"""CTC loss (T=512, B=32, C=8000, L=2, blank=0) on 8 Trainium2 NeuronCores.

Data-parallel over batch: each core takes 4 sequences. v5 structure:
  - host gathers the 16 needed logit streams per core as one contiguous
    [16, 512] tile X (a, y1, a_rev, y2_rev per sequence) -> tiny DMA,
  - ONE f32r PE matmul with a +-1 matrix Wm [16, 100] forms the DP
    streams at 32-aligned partition groups: d1 rows 0-3, d2rev rows
    32-35, y1 rows 64-67, a rows 96-99,
  - ONE fused inclusive cumsum (PBX [100, 513]); because d2 arrives
    time-reversed, PBX[32:36, 0:512] IS the suffix-sum P1brev and
    PBX[0:4, 0:512] IS P1a - no copies, no reversed tensor_scalar,
  - s1+s3 batched [36,512] as a direct view of PBX,
  - Vector carries only the critical chain (scans / maxes / P2 / Zthr);
    GpSimd carries the ZC/Zskip side chain and OUT staging copies,
  - device ships OUT [36,4] = (SZ, negMz, nm1, nm2); host does the
    final ln, max-unwinding, logaddexp, /L and batch mean in float64.

Notation (per sequence b, t = 0..511):
  a_t = logit[t,b,0], y1_t = logit[t,b,t1], y2_t = logit[t,b,t2]
  P1a_t = sum_{tau<t}(a-y1);  P1brev_c = sum_{t>511-c}(a-y2)
  W1 = ln cumsum exp(P1 - m1) + K;  P2rev_c = W1brev_{c-1} - P1brev_c
  W2 = ln cumsum exp(P2rev - m2) + K
  thr:  Zt_t = ZC_t + W2true_t       skip: Zs_t = ZC_{t+1} + P2true_t
  pcat0 = lnSZt + Mzt + m1a+m1b+m2 - 3K
  pcat1 = lnSZs + Mzs + m1a+m1b - 2K + skip
  loss_b = -logaddexp(pcat0, pcat1)/L
"""
import numpy as np

T = 512
B = 32
C = 8000
L = 2
NCORES = 8
BS = B // NCORES          # 4 sequences per core
XR = 4 * BS               # 16 input stream rows
NP = 100                  # stream partition span (groups at 0/32/64/96)
NZ = 36                   # two-group span (rows 0-3 and 32-35)
NEG = -1e30
EPS = 4.4e-20   # bottom edge of the HW Ln table's accurate range
KLN = 3e16      # scale so S*KLN spans the Ln-accurate domain
KAPPA = float(np.log(3e16))


def build_bass(dbg=False):
    import concourse.bass as bass
    import concourse.bacc as bacc
    import concourse.mybir as mybir
    import concourse.tile as tile
    from contextlib import ExitStack

    f32 = mybir.dt.float32
    f32r = mybir.dt.float32r
    AT = mybir.ActivationFunctionType
    OP = mybir.AluOpType
    AX = mybir.AxisListType

    nc = bacc.Bacc("TRN2", target_bir_lowering=False, debug=False,
                   num_devices=NCORES)

    # Exp and Ln share the natural_log_exp_and_others ACT table set; pin
    # the chooser there so the table loads once (no Exp<->Ln reloads).
    import types
    from concourse.hw_specs import get_activation_tables

    def _act_loads_one_set(self):
        has_activation = any(isinstance(i, mybir.InstActivation)
                             for b in self.main_func.blocks
                             for i in b.instructions)
        if not has_activation:
            return
        tables = [(n, (fns if n == "natural_log_exp_and_others" else set()))
                  for n, fns in get_activation_tables(self.m.arch).items()]
        bacc._bass_rust.insert_act_table_loads(self, tables)

    nc.insert_act_table_loads = types.MethodType(_act_loads_one_set, nc)

    x_ext = nc.dram_tensor("x", [XR, T], f32r, kind="ExternalInput")
    w_ext = nc.dram_tensor("w", [XR, NP], f32r, kind="ExternalInput")
    out_ext = nc.dram_tensor("out", [NZ, 4], f32, kind="ExternalOutput")

    with tile.TileContext(nc) as tc, ExitStack() as ctx:
        pool = ctx.enter_context(tc.tile_pool(name="p", bufs=1))
        ppool = ctx.enter_context(tc.tile_pool(name="ps", bufs=1, space="PSUM"))

        # ---------- DMAs first (both tiny and contiguous) ----------
        Xs = pool.tile([XR, T], f32r)
        Ws = pool.tile([XR, NP], f32r)
        nc.sync.dma_start(out=Xs[:], in_=x_ext[:])
        nc.scalar.dma_start(out=Ws[:], in_=w_ext[:])

        # ---------- constants + memsets (off critical path) ----------
        zeros = pool.tile([NP, 1], f32)
        nc.gpsimd.memset(zeros[:], 0.0)
        eps36 = pool.tile([NZ, 1], f32)
        nc.gpsimd.memset(eps36[:], EPS)

        PBX = ppool.tile([NP, T], f32, tag="PBX")
        nc.vector.memset(PBX[:, 0:1], 0.0)
        P2 = pool.tile([BS, T], f32)
        nc.gpsimd.memset(P2[:, 0:1], NEG)
        Z = pool.tile([NZ, T], f32)
        nc.gpsimd.memset(Z[0:BS, 0:1], NEG)
        nc.gpsimd.memset(Z[32:36, T - 1:T], NEG)

        # preload the Exp/Ln ACT table during the DMA window
        warm = pool.tile([1, 1], f32)
        nc.scalar.activation(warm[:], zeros[0:1, :], AT.Exp,
                             bias=eps36[0:1, :], scale=1.0)

        # ---------- phase A: one matmul -> streams [slot, t] ----------
        STR = ppool.tile([NP, T], f32, tag="STR")
        nc.tensor.matmul(STR[:], Ws[:], Xs[:], start=True, stop=True)

        # ---------- phase B: ONE fused inclusive cumsum ----------
        # PBX[r, t+1] = sum_{tau<=t} STR[r, tau]; col 0 = 0.
        # Rows 0-3: P1a = PBX[0:4, 0:512] (exclusive-view). Rows 32-35:
        # d2 arrives time-reversed, so P1brev = PBX[32:36, 0:512].
        nc.vector.tensor_tensor_scan(
            PBX[:, 1:T], STR[:, 0:T - 1],
            zeros[:].broadcast_to((NP, T - 1)), 0.0,
            op0=OP.add, op1=OP.bypass)
        P1v = PBX[0:NZ, 0:T]

        # ---------- stage s1 (rows 0-3) + s3 (rows 32-35, rev) ----------
        OUT = pool.tile([NZ, 4], f32)
        nm1 = pool.tile([NZ, 1], f32)
        nc.vector.tensor_reduce(nm1[:], P1v, axis=AX.X, op=OP.max,
                                negate=True)
        E1 = ppool.tile([NZ, T], f32, tag="E1")
        nc.scalar.activation(E1[:], P1v, AT.Exp, bias=nm1[:], scale=1.0)
        # PBX is PSUM: GpSimd cannot read it, so TY1z / ZCp run on
        # Vector inside the E1-exp and W1-ln windows; TAs_t (suffix-sum
        # of a) comes from the reversed a_rev cumsum rows 96-99.
        TY1z = pool.tile([BS, T], f32)    # TY1e at base partition 0
        ZCp = pool.tile([BS, T], f32)     # TY1e_t + TAs_t
        nc.vector.tensor_copy(TY1z[:], PBX[64:68, 0:T])
        nc.gpsimd.tensor_copy(OUT[:, 2:3], nm1[:])
        S1 = pool.tile([NZ, T], f32)
        nc.vector.tensor_tensor_scan(S1[:], E1[:],
                                     zeros[0:NZ, :].broadcast_to((NZ, T)),
                                     0.0, op0=OP.add, op1=OP.bypass)
        nc.vector.tensor_tensor(ZCp[:, 1:T], TY1z[:, 1:T],
                                PBX[96:100, 1:T][:, ::-1], op=OP.add)
        W1 = pool.tile([NZ, T], f32)      # W' = true W + KAPPA
        nc.scalar.activation(W1[:], S1[:], AT.Ln, bias=eps36[:], scale=KLN)

        # ---------- stage s2 (rev) ----------
        nc.vector.tensor_tensor(P2[:, 1:T], W1[32:36, 0:T - 1],
                                PBX[32:36, 1:T], op=OP.subtract)
        nm2 = pool.tile([BS, 1], f32)
        nc.vector.tensor_reduce(nm2[:], P2[:], axis=AX.X, op=OP.max,
                                negate=True)
        E2 = ppool.tile([BS, T], f32, tag="E2")
        nc.scalar.activation(E2[:], P2[:], AT.Exp, bias=nm2[:], scale=1.0)
        # ZC / Zskip side chain on GpSimd under the s2 windows
        ZC = pool.tile([BS, T], f32)      # ZC_t = TY1e_t + TAs_t + W1a_{t-1}
        nc.gpsimd.tensor_tensor(ZC[:, 1:T], ZCp[:, 1:T],
                                W1[0:BS, 0:T - 1], op=OP.add)
        nc.gpsimd.tensor_tensor(Z[32:36, 0:T - 1], ZC[:, 1:T],
                                P2[:, 1:T][:, ::-1], op=OP.add)
        nc.gpsimd.tensor_copy(OUT[0:BS, 3:4], nm2[:])
        S2 = pool.tile([BS, T], f32)
        nc.vector.tensor_tensor_scan(S2[:], E2[:],
                                     zeros[0:BS, :].broadcast_to((BS, T)),
                                     0.0, op0=OP.add, op1=OP.bypass)
        W2 = pool.tile([BS, T], f32)      # W' = true W + KAPPA
        nc.scalar.activation(W2[:], S2[:], AT.Ln, bias=eps36[0:BS, :],
                             scale=KLN)

        # ---------- combine: thr half, global max, EZ ----------
        nc.vector.tensor_tensor(Z[0:BS, 1:T], ZC[:, 1:T],
                                W2[:, 0:T - 1][:, ::-1], op=OP.add)
        negMz = pool.tile([NZ, 1], f32)
        nc.vector.tensor_reduce(negMz[:], Z[:], axis=AX.X, op=OP.max,
                                negate=True)
        nc.gpsimd.tensor_copy(OUT[:, 1:2], negMz[:])
        EZ = ppool.tile([NZ, T], f32, tag="EZ")
        SZ = pool.tile([NZ, 1], f32)
        nc.scalar.activation(EZ[:], Z[:], AT.Exp, bias=negMz[:], scale=1.0,
                             accum_out=SZ[:])
        nc.vector.tensor_copy(OUT[:, 0:1], SZ[:])
        nc.sync.dma_start(out=out_ext[:], in_=OUT[:])

    nc.compile()
    return nc


def make_in_maps(logit, targets):
    logit = np.asarray(logit, dtype=np.float32)
    targets = np.asarray(targets)
    in_maps = []
    for core in range(NCORES):
        tg = targets[core * BS:(core + 1) * BS]
        x = np.empty((XR, T), np.float32)
        for b in range(BS):
            gb = core * BS + b
            x[0 + b] = logit[:, gb, 0]                      # a
            x[BS + b] = logit[:, gb, int(tg[b, 0])]         # y1
            x[2 * BS + b] = logit[::-1, gb, 0]              # a reversed
            x[3 * BS + b] = logit[::-1, gb, int(tg[b, 1])]  # y2 reversed
        # group 96 stream switches to the REVERSED a (suffix sums)
        w = np.zeros((XR, NP), np.float32)
        for b in range(BS):
            w[0 + b, 0 + b] = 1.0        # d1 = a - y1
            w[BS + b, 0 + b] = -1.0
            w[2 * BS + b, 32 + b] = 1.0  # d2rev = a_rev - y2_rev
            w[3 * BS + b, 32 + b] = -1.0
            w[BS + b, 64 + b] = 1.0      # y1
            w[2 * BS + b, 96 + b] = 1.0  # a_rev (suffix sums)
        in_maps.append({"x": x, "w": w})
    return in_maps


def finish(results, targets):
    """Host gather: per-core OUT [36,4] -> per-seq losses [32] (float64)."""
    targets = np.asarray(targets)
    losses = np.empty(B, np.float64)
    for core, r in enumerate(results):
        o = np.asarray(r["out"], np.float64)     # [NZ, 4]
        sz_t, sz_s = o[0:BS, 0], o[32:36, 0]
        mz_t, mz_s = -o[0:BS, 1], -o[32:36, 1]
        m1a, m1b = -o[0:BS, 2], -o[32:36, 2]
        m2 = -o[0:BS, 3]
        tg = targets[core * BS:(core + 1) * BS]
        skip = np.where(tg[:, 0] != tg[:, 1], 0.0, NEG)
        pcat0 = np.log(sz_t) + mz_t + m1a + m1b + m2 - 3 * KAPPA
        pcat1 = np.log(sz_s) + mz_s + m1a + m1b - 2 * KAPPA + skip
        losses[core * BS:(core + 1) * BS] = \
            -np.logaddexp(pcat0, pcat1) / L
    return losses


_CACHED = {}


def kernel(logit, label, targets):
    from concourse.bass_utils import run_bass_kernel_spmd
    if "nc" not in _CACHED:
        _CACHED["nc"] = build_bass()
    nc = _CACHED["nc"]
    in_maps = make_in_maps(logit, targets)
    res = run_bass_kernel_spmd(nc, in_maps, core_ids=list(range(NCORES)))
    losses = finish(res.results, targets)
    return np.float32(losses.mean())
